# revision 22
# baseline (speedup 1.0000x reference)
"""Trainium2 Bass kernel for the LIIF-style guided upsampling MLP (nn_BF_NIR_conv).

Key structural insight: `grid_sample(nearest)` at the 4 shifted coords reduces to
parity-dependent integer shifts of the LR grid — for HR pixel (2k+p, 2l+q) and
neighbor (vx,vy)=(2a-1,2b-1), the sampled LR position is (k+p-1+a, l+q-1+b).
So we process pixels grouped by parity class (p,q); every "gather" becomes a
contiguous shifted window over a zero-padded LR feature slice, and `rel` is a
per-(class,neighbor) constant folded into the layer-1 bias (with small additive
fixup tiles for image-border pixels, where the reference's joint-validity rule
makes rel pixel-dependent).

Bilateral softmax weights: exp(D[dy,dx]) of the 9 shifted 3-channel dot maps and
the per-class softmax denominators are computed on the HOST and shipped
partition-replicated (bf16), so the device applies them as a single Pool-engine
multiply on h2 (weights are positive, and scaling commutes with the final
linear layer), accumulating the 4 neighbors' layer-3 outputs in one PSUM bank.

Sharding: core c handles HR rows [32c, 32c+32) — data-parallel over pixels, with
an 18-row LR halo slice instead of full replication.
"""
import numpy as np
from ml_dtypes import bfloat16

import concourse.bass as bass
import concourse.tile as tile
from concourse import mybir, bacc
from concourse.bass_utils import run_bass_kernel_spmd

F32 = mybir.dt.float32
BF16 = mybir.dt.bfloat16
AF = mybir.ActivationFunctionType
ALU = mybir.AluOpType
F32R = mybir.dt.float32r


def _r(ap):
    return ap.bitcast(F32R)

NCORES = 8
# combos enumerated as cmb = (2p+q)*4 + (2a+b)
ALL16 = [(p, q, a, b) for p in (0, 1) for q in (0, 1) for a in (0, 1) for b in (0, 1)]
ALL16 = sorted(ALL16, key=lambda t: ((2 * t[0] + t[1]) * 4 + 2 * t[2] + t[3]))
# col-border combos (l=0 col invalid for b=0&q=0; l=127 col invalid for b=1&q=1)
CB = [t for t in ALL16 if (t[1] == 0 and t[3] == 0) or (t[1] == 1 and t[3] == 1)]

# early-tensor packing (columns of the [128, NEARLY] bf16 tensor): matmul
# weights, biases and the first-needed fixups all ride one DMA
C_W1 = 0          # 512 (2 kb blocks of 256)
C_W1G = 512       # 256 guide block
C_W2 = 768        # 256 (2 blocks of 128)
C_CF = 1024       # 256 colfix
C_RF0 = 1280      # 1024 rowfix pat0
NEARLY = 2304
# f32 bias tensor (scalar operands must be f32)
C_B1 = 0          # 32
C_B2 = 32         # 1
C_B3 = 33         # 1 (rows 0:32)
NCONST = 34
NFIXB = 1024      # rowfix pat1, separate late tensor

_NC = None


def _build_nc():
    global _NC
    if _NC is not None:
        return _NC
    nc = bacc.Bacc("TRN2", target_bir_lowering=False)

    fc0 = nc.dram_tensor("fc0", [128, 18 * 130], BF16, kind="ExternalInput")
    fc1 = nc.dram_tensor("fc1", [128, 18 * 130], BF16, kind="ExternalInput")
    guide = nc.dram_tensor("guide", [128, 4 * 2048], BF16, kind="ExternalInput")
    early = nc.dram_tensor("early", [128, NEARLY], BF16, kind="ExternalInput")
    consts = nc.dram_tensor("consts", [128, NCONST], F32, kind="ExternalInput")
    fixesb = nc.dram_tensor("fixesb", [128, NFIXB], BF16, kind="ExternalInput")
    w3bf = nc.dram_tensor("w3bf", [128, 128], BF16, kind="ExternalInput")
    # host-computed bilateral weights: one line per LR-shift g=3u+v (the
    # (cls, neighbor) weight depends only on g), bf16, replicated across
    # all 128 partitions
    wrep = nc.dram_tensor("wrep", [128, 9 * 2048], BF16, kind="ExternalInput")
    # per-class softmax reciprocal, replicated across 32 partitions
    rrep = nc.dram_tensor("rrep", [32, 4 * 2048], BF16, kind="ExternalInput")
    # class-grouped output [32ch, cls, k, l] — contiguous stores; host de-interleaves
    y = nc.dram_tensor("y", [32, 4 * 2048], BF16, kind="ExternalOutput")

    with tile.TileContext(nc) as tc, \
         tc.tile_pool(name="const", bufs=1) as constp, \
         tc.tile_pool(name="work", bufs=3) as workp, \
         tc.tile_pool(name="outp", bufs=2) as outp, \
         tc.tile_pool(name="ph1", bufs=2, space="PSUM") as ph1, \
         tc.tile_pool(name="ph2", bufs=2, space="PSUM") as ph2, \
         tc.tile_pool(name="pop", bufs=2, space="PSUM") as pop:

        # ---- all loads dispatched up front, spread across the 3 DMA engines ----
        s_fc1 = constp.tile([128, 18 * 130], BF16)
        s_fc0 = constp.tile([128, 18 * 130], BF16)
        s_gd = [constp.tile([128, 2048], BF16, tag=f"gd{c}", name=f"gd{c}")
                for c in range(4)]
        s_early = constp.tile([128, NEARLY], BF16)
        s_consts = constp.tile([128, NCONST], F32)
        s_fixb = constp.tile([128, NFIXB], BF16)
        s_w3 = constp.tile([128, 128], BF16)
        s_w = [constp.tile([128, 2048], BF16, tag=f"w{g}", name=f"w{g}")
               for g in range(9)]
        s_r = constp.tile([32, 4 * 2048], BF16)

        # scalar (Act HWDGE): only the early gates — ACT's sequencer must be
        # free for compute once drains start
        nc.scalar.dma_start(out=s_early, in_=early[:, :])
        nc.scalar.dma_start(out=s_consts, in_=consts[:, :])
        nc.scalar.dma_start(out=s_w[0], in_=wrep[:, 0:2048])
        nc.scalar.dma_start(out=s_fixb, in_=fixesb[:, :])
        # sync (SP HWDGE): everything else, in consumption order
        nc.sync.dma_start(out=s_fc1, in_=fc1[:, :])
        nc.sync.dma_start(out=s_fc0, in_=fc0[:, :])
        nc.sync.dma_start(out=s_gd[0], in_=guide[:, 0:2048])
        nc.sync.dma_start(out=s_w3, in_=w3bf[:, :])
        for g in (1, 3, 4):  # rest of cls0's lines
            nc.sync.dma_start(out=s_w[g], in_=wrep[:, 2048 * g:2048 * (g + 1)])
        nc.sync.dma_start(out=s_r, in_=rrep[:, :])
        nc.sync.dma_start(out=s_gd[1], in_=guide[:, 2048:4096])
        for g in (2, 5):  # cls1 adds g2, g5
            nc.sync.dma_start(out=s_w[g], in_=wrep[:, 2048 * g:2048 * (g + 1)])
        nc.sync.dma_start(out=s_gd[2], in_=guide[:, 4096:6144])
        for g in (6, 7):  # cls2 adds g6, g7
            nc.sync.dma_start(out=s_w[g], in_=wrep[:, 2048 * g:2048 * (g + 1)])
        nc.sync.dma_start(out=s_gd[3], in_=guide[:, 6144:8192])
        nc.sync.dma_start(out=s_w[8], in_=wrep[:, 8 * 2048:9 * 2048])

        # PE pre-warm: dummy matmuls with no DMA dependencies ramp the PE
        # clock out of its low P-state while the real loads are in flight
        warm = constp.tile([128, 512], BF16, name="warm")
        nc.vector.memset(warm[:, :], 0)
        for i in range(12):
            wps = ph1.tile([128, 512], F32, tag="h1ps0", name="h1ps0")
            nc.tensor.matmul(wps[:, :], warm[:, 0:128], warm[:, :],
                             start=True, stop=True)

        fc0r = s_fc0[:, :].rearrange("c (r x) -> c r x", x=130)
        fc1r = s_fc1[:, :].rearrange("c (r x) -> c r x", x=130)
        lw1 = lambda kb, blk: s_early[:, C_W1 + kb * 256 + blk * 128:
                                      C_W1 + kb * 256 + blk * 128 + 128]
        lw2 = lambda blk: s_early[:, C_W2 + blk * 128:C_W2 + blk * 128 + 128]

        # ---- main per-(class, chunk, neighbor) pipeline ----
        for cls in range(4):
            p, q = cls >> 1, cls & 1
            for ck in range(4):
                opred = pop.tile([128, 512], F32, tag="opred")
                for j in range(4):
                    a, b = j >> 1, j & 1
                    cmb = cls * 4 + j
                    h1ps = [ph1.tile([128, 512], F32, tag=f"h1ps{blk}",
                                     name=f"h1ps{blk}") for blk in range(2)]
                    for blk in range(2):
                        ps = h1ps[blk][:, :]
                        rs, cs = 4 * ck + p + a, q + b
                        nc.tensor.matmul(ps, lw1(1, blk),
                                         fc1r[:, rs:rs + 4, cs:cs + 128],
                                         start=True, stop=False)
                        nc.tensor.matmul(ps, lw1(0, blk),
                                         fc0r[:, rs:rs + 4, cs:cs + 128],
                                         start=False, stop=False)
                        nc.tensor.matmul(ps, s_early[:, C_W1G + blk * 128:C_W1G + blk * 128 + 128],
                                         s_gd[cls][:, 512 * ck:512 * (ck + 1)],
                                         start=False, stop=True)
                    # border fixups (pre-relu)
                    if (q == 0 and b == 0) or (q == 1 and b == 1):
                        ci = CB.index((p, q, a, b))
                        l0 = 0 if q == 0 else 127
                        for blk in range(2):
                            view = h1ps[blk][:, l0::128]
                            fx = s_early[:, C_CF + (ci * 2 + blk) * 16 + 4 * ck:
                                          C_CF + (ci * 2 + blk) * 16 + 4 * ck + 4]
                            nc.vector.tensor_add(view, view, fx)
                    if (p, a) == (0, 0) and ck == 0:
                        ri = 2 * q + b
                        for blk in range(2):
                            view = h1ps[blk][:, 0:128]
                            base = C_RF0 + (ri * 2 + blk) * 128
                            nc.vector.tensor_add(view, view,
                                                 s_early[:, base:base + 128])
                    if (p, a) == (1, 1) and ck == 3:
                        ri = 2 * q + b
                        for blk in range(2):
                            view = h1ps[blk][:, 384:512]
                            base = (ri * 2 + blk) * 128
                            nc.vector.tensor_add(view, view,
                                                 s_fixb[:, base:base + 128])
                    # relu + bias -> SBUF (split across ACT and DVE)
                    h1sb = [workp.tile([128, 512], BF16, tag=f"h1sb{blk}",
                                       name=f"h1sb{blk}") for blk in range(2)]
                    nc.scalar.activation(h1sb[0][:, :], h1ps[0][:, :], AF.Relu,
                                         bias=s_consts[:, C_B1 + cmb * 2:
                                                       C_B1 + cmb * 2 + 1])
                    nc.vector.tensor_scalar(h1sb[1][:, :], h1ps[1][:, :],
                                            s_consts[:, C_B1 + cmb * 2 + 1:
                                                     C_B1 + cmb * 2 + 2],
                                            0.0, ALU.add, ALU.max)
                    # layer 2
                    h2ps = ph2.tile([128, 512], F32, tag="h2ps")
                    nc.tensor.matmul(h2ps[:, :], lw2(0), h1sb[0][:, :],
                                     start=True, stop=False)
                    nc.tensor.matmul(h2ps[:, :], lw2(1), h1sb[1][:, :],
                                     start=False, stop=True)
                    h2sb = workp.tile([128, 512], BF16, tag="h2sb")
                    nc.scalar.activation(h2sb[:, :], h2ps[:, :], AF.Relu,
                                         bias=s_consts[:, C_B2:C_B2 + 1])
                    # bilateral weighting on the Pool engine (weights > 0, and a
                    # per-pixel scale commutes with layer 3)
                    h2w = workp.tile([128, 512], BF16, tag="h2w")
                    g9 = 3 * (p + a) + (q + b)
                    nc.vector.tensor_mul(h2w[:, :], h2sb[:, :],
                                         s_w[g9][:, 512 * ck:512 * (ck + 1)])
                    # layer 3, accumulating the 4 neighbors into one psum
                    nc.tensor.matmul(opred[:, :], s_w3[:, :], h2w[:, :],
                                     start=(j == 0), stop=(j == 3),
                                     skip_group_check=True)
                # normalize by 1/sum(w) and add b3
                osb = outp.tile([32, 512], F32, tag="osb")
                nc.vector.tensor_mul(osb[:, :], opred[0:32, :],
                                     s_r[:, 2048 * cls + 512 * ck:
                                         2048 * cls + 512 * (ck + 1)])
                osbh = outp.tile([32, 512], BF16, tag="osbh")
                nc.scalar.activation(osbh[:, :], osb[:, :], AF.Identity,
                                     bias=s_consts[0:32, C_B3:C_B3 + 1])
                nc.sync.dma_start(
                    out=y[:, 2048 * cls + 512 * ck:2048 * cls + 512 * (ck + 1)],
                    in_=osbh[:, :])

    nc.compile()
    _NC = nc
    return nc


def _prep_core(c, feat, lr_guide, hr_guide, W1, b1, W2, b2, W3, b3):
    def pad_slice(img):  # [128, 128, 128] -> [128, 18, 130] zero-padded halo
        out = np.zeros((128, 18, 130), np.float32)
        y0 = 16 * c - 1
        ys, ye = max(y0, 0), min(16 * c + 17, 128)
        out[:, ys - y0:ye - y0, 1:129] = img[:, ys:ye, :]
        return out.reshape(128, 18 * 130)

    fc0 = pad_slice(lr_guide[0]).astype(bfloat16)
    fc1 = pad_slice(feat[0]).astype(bfloat16)
    strip = hr_guide[0][:, 32 * c:32 * c + 32, :]
    g = np.empty((128, 4, 16, 128), np.float32)
    for p in range(2):
        for q in range(2):
            g[:, 2 * p + q] = strip[:, p::2, q::2]

    W1y, W1x = W1[384], W1[385]
    bias1 = np.zeros((128, 32), np.float32)
    for cmb, (p, q, a, b) in enumerate(ALL16):
        v = b1 + (1.5 - p - 2 * a) * W1y + (1.5 - q - 2 * b) * W1x
        bias1[:, cmb * 2] = v[:128]
        bias1[:, cmb * 2 + 1] = v[128:]

    colfix = np.zeros((128, 256), np.float32)
    for ci, (p, q, a, b) in enumerate(CB):
        l0 = 0 if q == 0 else 127
        relx_inv = (2 * l0 + q) + 0.5 - 128.0
        relx_int = 1.5 - q - 2 * b
        rely_int = 1.5 - p - 2 * a
        for k in range(16):
            I = 32 * c + 2 * k + p
            d = (I + 0.5 - 128.0 - rely_int) * W1y + (relx_inv - relx_int) * W1x
            if c == 0 and (p, a) == (0, 0) and k == 0:
                d = 0 * d
            if c == 7 and (p, a) == (1, 1) and k == 15:
                d = 0 * d
            colfix[:, (ci * 2 + 0) * 16 + k] = d[:128]
            colfix[:, (ci * 2 + 1) * 16 + k] = d[128:]

    rowfix = np.zeros((128, 2048), np.float32)
    for pat in range(2):
        if (pat == 0 and c != 0) or (pat == 1 and c != 7):
            continue
        p = a = pat
        k = 0 if pat == 0 else 15
        I = 32 * c + 2 * k + p
        rely_inv = I + 0.5 - 128.0
        rely_int = 1.5 - p - 2 * a
        for ri, (q, b) in enumerate([(0, 0), (0, 1), (1, 0), (1, 1)]):
            relx_int = 1.5 - q - 2 * b
            J = 2 * np.arange(128, dtype=np.float32) + q
            relx_inv = J + 0.5 - 128.0
            d = (rely_inv - rely_int) * W1y[:, None] + \
                np.outer(W1x, relx_inv - relx_int)  # [256, 128]
            base0 = ((pat * 4 + ri) * 2 + 0) * 128
            base1 = ((pat * 4 + ri) * 2 + 1) * 128
            rowfix[:, base0:base0 + 128] = d[:128]
            rowfix[:, base1:base1 + 128] = d[128:]

    early = np.zeros((128, NEARLY), np.float32)
    early[:, C_W1:C_W1 + 512] = np.stack([W1[0:128], W1[128:256]],
                                         axis=1).reshape(128, 512)
    early[:, C_W1G:C_W1G + 256] = W1[256:384]
    early[:, C_W2:C_W2 + 256] = np.stack([W2[0:128], W2[128:256]],
                                         axis=1).reshape(128, 256)
    early[:, C_CF:C_CF + 256] = colfix
    early[:, C_RF0:C_RF0 + 1024] = rowfix[:, 0:1024]
    fixesb = rowfix[:, 1024:2048]
    consts = np.zeros((128, NCONST), np.float32)
    consts[:, C_B1:C_B1 + 32] = bias1
    consts[:, C_B2] = b2
    consts[:32, C_B3] = b3

    # bilateral weights: D[u,v] = 3-channel dot of center LR cell with the
    # (u-1, v-1)-shifted cell (zero padded), channels feat[124:127]
    ch = feat[0, 124:127]  # [3, 128, 128]
    chp = np.zeros((3, 130, 130), np.float32)
    chp[:, 1:129, 1:129] = ch
    e = np.empty((3, 3, 128, 128), np.float32)
    for u in range(3):
        for v in range(3):
            D = (chp[:, u:u + 128, v:v + 128] * ch).sum(axis=0)
            e[u, v] = np.exp(D)
    k0 = 16 * c
    wrep_line = np.empty((9, 2048), np.float32)
    rrep_line = np.empty((4, 2048), np.float32)
    for gi in range(9):
        wrep_line[gi] = e[gi // 3, gi % 3][k0:k0 + 16].reshape(2048)
    for cls in range(4):
        p, q = cls >> 1, cls & 1
        s = (e[p, q] + e[p, q + 1] + e[p + 1, q] + e[p + 1, q + 1])
        rrep_line[cls] = (1.0 / s)[k0:k0 + 16].reshape(2048)
    wrep = np.broadcast_to(wrep_line.reshape(1, 9 * 2048).astype(bfloat16),
                           (128, 9 * 2048))
    rrep = np.broadcast_to(rrep_line.reshape(1, 4 * 2048).astype(bfloat16),
                           (32, 4 * 2048))

    return {
        "fc0": fc0, "fc1": fc1,
        "guide": np.ascontiguousarray(g.reshape(128, 8192).astype(bfloat16)),
        "early": np.ascontiguousarray(early.astype(bfloat16)),
        "consts": consts,
        "fixesb": np.ascontiguousarray(fixesb.astype(bfloat16)),
        "w3bf": np.ascontiguousarray(np.pad(W3, ((0, 0), (0, 96))).astype(bfloat16)),
        "wrep": np.ascontiguousarray(wrep),
        "rrep": np.ascontiguousarray(rrep),
    }


def kernel(**inputs):
    feat = np.asarray(inputs["feat"], np.float32)
    lr_guide = np.asarray(inputs["lr_guide"], np.float32)
    hr_guide = np.asarray(inputs["hr_guide"], np.float32)
    W1 = np.asarray(inputs["W1"], np.float32)
    b1 = np.asarray(inputs["b1"], np.float32)
    W2 = np.asarray(inputs["W2"], np.float32)
    b2 = np.asarray(inputs["b2"], np.float32)
    W3 = np.asarray(inputs["W3"], np.float32)
    b3 = np.asarray(inputs["b3"], np.float32)

    nc = _build_nc()
    in_maps = [_prep_core(c, feat, lr_guide, hr_guide, W1, b1, W2, b2, W3, b3)
               for c in range(NCORES)]
    res = run_bass_kernel_spmd(nc, in_maps, core_ids=list(range(NCORES)))
    out = np.zeros((1, 32, 256, 256), np.float32)
    for c in range(NCORES):
        yc = np.asarray(res.results[c]["y"],
                        dtype=np.float32).reshape(32, 4, 16, 128)
        strip = out[0, :, 32 * c:32 * c + 32, :]
        for p in range(2):
            for q in range(2):
                strip[:, p::2, q::2] = yc[:, 2 * p + q]
    return out


# revision 23
# speedup vs baseline: 1.2024x; 1.2024x over previous
"""Trainium2 Bass kernel for the LIIF-style guided upsampling MLP (nn_BF_NIR_conv).

Key structural insight: `grid_sample(nearest)` at the 4 shifted coords reduces to
parity-dependent integer shifts of the LR grid — for HR pixel (2k+p, 2l+q) and
neighbor (vx,vy)=(2a-1,2b-1), the sampled LR position is (k+p-1+a, l+q-1+b).
So we process pixels grouped by parity class (p,q); every "gather" becomes a
contiguous shifted window over a zero-padded LR feature slice, and `rel` is a
per-(class,neighbor) constant folded into the layer-1 bias (with small additive
fixup tiles for image-border pixels, where the reference's joint-validity rule
makes rel pixel-dependent).

Bilateral softmax weights: exp(D[dy,dx]) of the 9 shifted 3-channel dot maps and
the per-class softmax denominators are computed on the HOST and shipped
partition-replicated (bf16), so the device applies them as a single Pool-engine
multiply on h2 (weights are positive, and scaling commutes with the final
linear layer), accumulating the 4 neighbors' layer-3 outputs in one PSUM bank.

Sharding: core c handles HR rows [32c, 32c+32) — data-parallel over pixels, with
an 18-row LR halo slice instead of full replication.
"""
import numpy as np
from ml_dtypes import bfloat16

import concourse.bass as bass
import concourse.tile as tile
from concourse import mybir, bacc
from concourse.bass_utils import run_bass_kernel_spmd

F32 = mybir.dt.float32
BF16 = mybir.dt.bfloat16
AF = mybir.ActivationFunctionType
ALU = mybir.AluOpType
F32R = mybir.dt.float32r


def _r(ap):
    return ap.bitcast(F32R)

NCORES = 8
# combos enumerated as cmb = (2p+q)*4 + (2a+b)
ALL16 = [(p, q, a, b) for p in (0, 1) for q in (0, 1) for a in (0, 1) for b in (0, 1)]
ALL16 = sorted(ALL16, key=lambda t: ((2 * t[0] + t[1]) * 4 + 2 * t[2] + t[3]))
# col-border combos (l=0 col invalid for b=0&q=0; l=127 col invalid for b=1&q=1)
CB = [t for t in ALL16 if (t[1] == 0 and t[3] == 0) or (t[1] == 1 and t[3] == 1)]

# compact matmul-weight tensor (small row stride keeps LDWEIGHTS fast)
C_W1 = 0          # 512 (2 kb blocks of 256)
C_W1G = 512       # 256 guide block
C_W2 = 768        # 256 (2 blocks of 128)
NWTS = 1024
# first-needed fixups
C_CF = 0          # 256 colfix
C_RF0 = 256       # 1024 rowfix pat0
NEARLY = 1280
# f32 bias tensor (scalar operands must be f32)
C_B1 = 0          # 32
C_B2 = 32         # 1
C_B3 = 33         # 1 (rows 0:32)
NCONST = 34
NFIXB = 1024      # rowfix pat1, separate late tensor

_NC = None


def _build_nc():
    global _NC
    if _NC is not None:
        return _NC
    nc = bacc.Bacc("TRN2", target_bir_lowering=False)

    fc0 = nc.dram_tensor("fc0", [128, 18 * 130], BF16, kind="ExternalInput")
    fc1 = nc.dram_tensor("fc1", [128, 18 * 130], BF16, kind="ExternalInput")
    guide = nc.dram_tensor("guide", [128, 4 * 2048], BF16, kind="ExternalInput")
    wts = nc.dram_tensor("wts", [128, NWTS], BF16, kind="ExternalInput")
    early = nc.dram_tensor("early", [128, NEARLY], BF16, kind="ExternalInput")
    consts = nc.dram_tensor("consts", [128, NCONST], F32, kind="ExternalInput")
    fixesb = nc.dram_tensor("fixesb", [128, NFIXB], BF16, kind="ExternalInput")
    w3bf = nc.dram_tensor("w3bf", [128, 128], BF16, kind="ExternalInput")
    # host-computed bilateral weights: one line per LR-shift g=3u+v (the
    # (cls, neighbor) weight depends only on g), bf16, replicated across
    # all 128 partitions
    wrep = nc.dram_tensor("wrep", [128, 9 * 2048], BF16, kind="ExternalInput")
    # per-class softmax reciprocal, replicated across 32 partitions
    rrep = nc.dram_tensor("rrep", [32, 4 * 2048], BF16, kind="ExternalInput")
    # class-grouped output [32ch, cls, k, l] — contiguous stores; host de-interleaves
    y = nc.dram_tensor("y", [32, 4 * 2048], BF16, kind="ExternalOutput")

    with tile.TileContext(nc) as tc, \
         tc.tile_pool(name="const", bufs=1) as constp, \
         tc.tile_pool(name="work", bufs=3) as workp, \
         tc.tile_pool(name="outp", bufs=2) as outp, \
         tc.tile_pool(name="ph1", bufs=2, space="PSUM") as ph1, \
         tc.tile_pool(name="ph2", bufs=2, space="PSUM") as ph2, \
         tc.tile_pool(name="pop", bufs=2, space="PSUM") as pop:

        # ---- all loads dispatched up front, spread across the 3 DMA engines ----
        s_fc1 = constp.tile([128, 18 * 130], BF16)
        s_fc0 = constp.tile([128, 18 * 130], BF16)
        s_gd = [constp.tile([128, 2048], BF16, tag=f"gd{c}", name=f"gd{c}")
                for c in range(4)]
        s_wts = constp.tile([128, NWTS], BF16)
        s_early = constp.tile([128, NEARLY], BF16)
        s_consts = constp.tile([128, NCONST], F32)
        s_fixb = constp.tile([128, NFIXB], BF16)
        s_w3 = constp.tile([128, 128], BF16)
        s_w = [constp.tile([128, 2048], BF16, tag=f"w{g}", name=f"w{g}")
               for g in range(9)]
        s_r = constp.tile([32, 4 * 2048], BF16)

        # scalar (Act HWDGE): only the early gates — ACT's sequencer must be
        # free for compute once drains start
        nc.scalar.dma_start(out=s_wts, in_=wts[:, :])
        nc.scalar.dma_start(out=s_early, in_=early[:, :])
        nc.scalar.dma_start(out=s_consts, in_=consts[:, :])
        nc.scalar.dma_start(out=s_w[0], in_=wrep[:, 0:2048])
        nc.scalar.dma_start(out=s_fixb, in_=fixesb[:, :])
        # sync (SP HWDGE): everything else, in consumption order
        nc.sync.dma_start(out=s_fc1, in_=fc1[:, :])
        nc.sync.dma_start(out=s_fc0, in_=fc0[:, :])
        nc.sync.dma_start(out=s_gd[0], in_=guide[:, 0:2048])
        nc.sync.dma_start(out=s_w3, in_=w3bf[:, :])
        for g in (1, 3, 4):  # rest of cls0's lines
            nc.sync.dma_start(out=s_w[g], in_=wrep[:, 2048 * g:2048 * (g + 1)])
        nc.sync.dma_start(out=s_r, in_=rrep[:, :])
        nc.sync.dma_start(out=s_gd[1], in_=guide[:, 2048:4096])
        for g in (2, 5):  # cls1 adds g2, g5
            nc.sync.dma_start(out=s_w[g], in_=wrep[:, 2048 * g:2048 * (g + 1)])
        nc.sync.dma_start(out=s_gd[2], in_=guide[:, 4096:6144])
        for g in (6, 7):  # cls2 adds g6, g7
            nc.sync.dma_start(out=s_w[g], in_=wrep[:, 2048 * g:2048 * (g + 1)])
        nc.sync.dma_start(out=s_gd[3], in_=guide[:, 6144:8192])
        nc.sync.dma_start(out=s_w[8], in_=wrep[:, 8 * 2048:9 * 2048])

        # PE pre-warm: dummy matmuls with no DMA dependencies ramp the PE
        # clock out of its low P-state while the real loads are in flight
        warm = constp.tile([128, 512], BF16, name="warm")
        nc.vector.memset(warm[:, :], 0)
        for i in range(12):
            wps = ph1.tile([128, 512], F32, tag="h1ps0", name="h1ps0")
            nc.tensor.matmul(wps[:, :], warm[:, 0:128], warm[:, :],
                             start=True, stop=True)

        fc0r = s_fc0[:, :].rearrange("c (r x) -> c r x", x=130)
        fc1r = s_fc1[:, :].rearrange("c (r x) -> c r x", x=130)
        lw1 = lambda kb, blk: s_wts[:, C_W1 + kb * 256 + blk * 128:
                                    C_W1 + kb * 256 + blk * 128 + 128]
        lw2 = lambda blk: s_wts[:, C_W2 + blk * 128:C_W2 + blk * 128 + 128]

        # ---- main per-(class, chunk, neighbor) pipeline ----
        for cls in range(4):
            p, q = cls >> 1, cls & 1
            for ck in range(4):
                opred = pop.tile([128, 512], F32, tag="opred")
                for j in range(4):
                    a, b = j >> 1, j & 1
                    cmb = cls * 4 + j
                    h1ps = [ph1.tile([128, 512], F32, tag=f"h1ps{blk}",
                                     name=f"h1ps{blk}") for blk in range(2)]
                    for blk in range(2):
                        ps = h1ps[blk][:, :]
                        rs, cs = 4 * ck + p + a, q + b
                        nc.tensor.matmul(ps, lw1(1, blk),
                                         fc1r[:, rs:rs + 4, cs:cs + 128],
                                         start=True, stop=False)
                        nc.tensor.matmul(ps, lw1(0, blk),
                                         fc0r[:, rs:rs + 4, cs:cs + 128],
                                         start=False, stop=False)
                        nc.tensor.matmul(ps, s_wts[:, C_W1G + blk * 128:C_W1G + blk * 128 + 128],
                                         s_gd[cls][:, 512 * ck:512 * (ck + 1)],
                                         start=False, stop=True)
                    # border fixups (pre-relu)
                    if (q == 0 and b == 0) or (q == 1 and b == 1):
                        ci = CB.index((p, q, a, b))
                        l0 = 0 if q == 0 else 127
                        for blk in range(2):
                            view = h1ps[blk][:, l0::128]
                            fx = s_early[:, C_CF + (ci * 2 + blk) * 16 + 4 * ck:
                                          C_CF + (ci * 2 + blk) * 16 + 4 * ck + 4]
                            nc.vector.tensor_add(view, view, fx)
                    if (p, a) == (0, 0) and ck == 0:
                        ri = 2 * q + b
                        for blk in range(2):
                            view = h1ps[blk][:, 0:128]
                            base = C_RF0 + (ri * 2 + blk) * 128
                            nc.vector.tensor_add(view, view,
                                                 s_early[:, base:base + 128])
                    if (p, a) == (1, 1) and ck == 3:
                        ri = 2 * q + b
                        for blk in range(2):
                            view = h1ps[blk][:, 384:512]
                            base = (ri * 2 + blk) * 128
                            nc.vector.tensor_add(view, view,
                                                 s_fixb[:, base:base + 128])
                    # relu + bias -> SBUF (split across ACT and DVE)
                    h1sb = [workp.tile([128, 512], BF16, tag=f"h1sb{blk}",
                                       name=f"h1sb{blk}") for blk in range(2)]
                    nc.scalar.activation(h1sb[0][:, :], h1ps[0][:, :], AF.Relu,
                                         bias=s_consts[:, C_B1 + cmb * 2:
                                                       C_B1 + cmb * 2 + 1])
                    nc.vector.tensor_scalar(h1sb[1][:, :], h1ps[1][:, :],
                                            s_consts[:, C_B1 + cmb * 2 + 1:
                                                     C_B1 + cmb * 2 + 2],
                                            0.0, ALU.add, ALU.max)
                    # layer 2
                    h2ps = ph2.tile([128, 512], F32, tag="h2ps")
                    nc.tensor.matmul(h2ps[:, :], lw2(0), h1sb[0][:, :],
                                     start=True, stop=False)
                    nc.tensor.matmul(h2ps[:, :], lw2(1), h1sb[1][:, :],
                                     start=False, stop=True)
                    h2sb = workp.tile([128, 512], BF16, tag="h2sb")
                    nc.scalar.activation(h2sb[:, :], h2ps[:, :], AF.Relu,
                                         bias=s_consts[:, C_B2:C_B2 + 1])
                    # bilateral weighting on the Pool engine (weights > 0, and a
                    # per-pixel scale commutes with layer 3)
                    h2w = workp.tile([128, 512], BF16, tag="h2w")
                    g9 = 3 * (p + a) + (q + b)
                    nc.vector.tensor_mul(h2w[:, :], h2sb[:, :],
                                         s_w[g9][:, 512 * ck:512 * (ck + 1)])
                    # layer 3, accumulating the 4 neighbors into one psum
                    nc.tensor.matmul(opred[:, :], s_w3[:, :], h2w[:, :],
                                     start=(j == 0), stop=(j == 3),
                                     skip_group_check=True)
                # normalize by 1/sum(w) and add b3
                osb = outp.tile([32, 512], F32, tag="osb")
                nc.vector.tensor_mul(osb[:, :], opred[0:32, :],
                                     s_r[:, 2048 * cls + 512 * ck:
                                         2048 * cls + 512 * (ck + 1)])
                osbh = outp.tile([32, 512], BF16, tag="osbh")
                nc.scalar.activation(osbh[:, :], osb[:, :], AF.Identity,
                                     bias=s_consts[0:32, C_B3:C_B3 + 1])
                nc.sync.dma_start(
                    out=y[:, 2048 * cls + 512 * ck:2048 * cls + 512 * (ck + 1)],
                    in_=osbh[:, :])

    nc.compile()
    _NC = nc
    return nc


def _prep_core(c, feat, lr_guide, hr_guide, W1, b1, W2, b2, W3, b3):
    def pad_slice(img):  # [128, 128, 128] -> [128, 18, 130] zero-padded halo
        out = np.zeros((128, 18, 130), np.float32)
        y0 = 16 * c - 1
        ys, ye = max(y0, 0), min(16 * c + 17, 128)
        out[:, ys - y0:ye - y0, 1:129] = img[:, ys:ye, :]
        return out.reshape(128, 18 * 130)

    fc0 = pad_slice(lr_guide[0]).astype(bfloat16)
    fc1 = pad_slice(feat[0]).astype(bfloat16)
    strip = hr_guide[0][:, 32 * c:32 * c + 32, :]
    g = np.empty((128, 4, 16, 128), np.float32)
    for p in range(2):
        for q in range(2):
            g[:, 2 * p + q] = strip[:, p::2, q::2]

    W1y, W1x = W1[384], W1[385]
    bias1 = np.zeros((128, 32), np.float32)
    for cmb, (p, q, a, b) in enumerate(ALL16):
        v = b1 + (1.5 - p - 2 * a) * W1y + (1.5 - q - 2 * b) * W1x
        bias1[:, cmb * 2] = v[:128]
        bias1[:, cmb * 2 + 1] = v[128:]

    colfix = np.zeros((128, 256), np.float32)
    for ci, (p, q, a, b) in enumerate(CB):
        l0 = 0 if q == 0 else 127
        relx_inv = (2 * l0 + q) + 0.5 - 128.0
        relx_int = 1.5 - q - 2 * b
        rely_int = 1.5 - p - 2 * a
        for k in range(16):
            I = 32 * c + 2 * k + p
            d = (I + 0.5 - 128.0 - rely_int) * W1y + (relx_inv - relx_int) * W1x
            if c == 0 and (p, a) == (0, 0) and k == 0:
                d = 0 * d
            if c == 7 and (p, a) == (1, 1) and k == 15:
                d = 0 * d
            colfix[:, (ci * 2 + 0) * 16 + k] = d[:128]
            colfix[:, (ci * 2 + 1) * 16 + k] = d[128:]

    rowfix = np.zeros((128, 2048), np.float32)
    for pat in range(2):
        if (pat == 0 and c != 0) or (pat == 1 and c != 7):
            continue
        p = a = pat
        k = 0 if pat == 0 else 15
        I = 32 * c + 2 * k + p
        rely_inv = I + 0.5 - 128.0
        rely_int = 1.5 - p - 2 * a
        for ri, (q, b) in enumerate([(0, 0), (0, 1), (1, 0), (1, 1)]):
            relx_int = 1.5 - q - 2 * b
            J = 2 * np.arange(128, dtype=np.float32) + q
            relx_inv = J + 0.5 - 128.0
            d = (rely_inv - rely_int) * W1y[:, None] + \
                np.outer(W1x, relx_inv - relx_int)  # [256, 128]
            base0 = ((pat * 4 + ri) * 2 + 0) * 128
            base1 = ((pat * 4 + ri) * 2 + 1) * 128
            rowfix[:, base0:base0 + 128] = d[:128]
            rowfix[:, base1:base1 + 128] = d[128:]

    wts = np.zeros((128, NWTS), np.float32)
    wts[:, C_W1:C_W1 + 512] = np.stack([W1[0:128], W1[128:256]],
                                       axis=1).reshape(128, 512)
    wts[:, C_W1G:C_W1G + 256] = W1[256:384]
    wts[:, C_W2:C_W2 + 256] = np.stack([W2[0:128], W2[128:256]],
                                       axis=1).reshape(128, 256)
    early = np.zeros((128, NEARLY), np.float32)
    early[:, C_CF:C_CF + 256] = colfix
    early[:, C_RF0:C_RF0 + 1024] = rowfix[:, 0:1024]
    fixesb = rowfix[:, 1024:2048]
    consts = np.zeros((128, NCONST), np.float32)
    consts[:, C_B1:C_B1 + 32] = bias1
    consts[:, C_B2] = b2
    consts[:32, C_B3] = b3

    # bilateral weights: D[u,v] = 3-channel dot of center LR cell with the
    # (u-1, v-1)-shifted cell (zero padded), channels feat[124:127]
    ch = feat[0, 124:127]  # [3, 128, 128]
    chp = np.zeros((3, 130, 130), np.float32)
    chp[:, 1:129, 1:129] = ch
    e = np.empty((3, 3, 128, 128), np.float32)
    for u in range(3):
        for v in range(3):
            D = (chp[:, u:u + 128, v:v + 128] * ch).sum(axis=0)
            e[u, v] = np.exp(D)
    k0 = 16 * c
    wrep_line = np.empty((9, 2048), np.float32)
    rrep_line = np.empty((4, 2048), np.float32)
    for gi in range(9):
        wrep_line[gi] = e[gi // 3, gi % 3][k0:k0 + 16].reshape(2048)
    for cls in range(4):
        p, q = cls >> 1, cls & 1
        s = (e[p, q] + e[p, q + 1] + e[p + 1, q] + e[p + 1, q + 1])
        rrep_line[cls] = (1.0 / s)[k0:k0 + 16].reshape(2048)
    wrep = np.broadcast_to(wrep_line.reshape(1, 9 * 2048).astype(bfloat16),
                           (128, 9 * 2048))
    rrep = np.broadcast_to(rrep_line.reshape(1, 4 * 2048).astype(bfloat16),
                           (32, 4 * 2048))

    return {
        "fc0": fc0, "fc1": fc1,
        "guide": np.ascontiguousarray(g.reshape(128, 8192).astype(bfloat16)),
        "wts": np.ascontiguousarray(wts.astype(bfloat16)),
        "early": np.ascontiguousarray(early.astype(bfloat16)),
        "consts": consts,
        "fixesb": np.ascontiguousarray(fixesb.astype(bfloat16)),
        "w3bf": np.ascontiguousarray(np.pad(W3, ((0, 0), (0, 96))).astype(bfloat16)),
        "wrep": np.ascontiguousarray(wrep),
        "rrep": np.ascontiguousarray(rrep),
    }


def kernel(**inputs):
    feat = np.asarray(inputs["feat"], np.float32)
    lr_guide = np.asarray(inputs["lr_guide"], np.float32)
    hr_guide = np.asarray(inputs["hr_guide"], np.float32)
    W1 = np.asarray(inputs["W1"], np.float32)
    b1 = np.asarray(inputs["b1"], np.float32)
    W2 = np.asarray(inputs["W2"], np.float32)
    b2 = np.asarray(inputs["b2"], np.float32)
    W3 = np.asarray(inputs["W3"], np.float32)
    b3 = np.asarray(inputs["b3"], np.float32)

    nc = _build_nc()
    in_maps = [_prep_core(c, feat, lr_guide, hr_guide, W1, b1, W2, b2, W3, b3)
               for c in range(NCORES)]
    res = run_bass_kernel_spmd(nc, in_maps, core_ids=list(range(NCORES)))
    out = np.zeros((1, 32, 256, 256), np.float32)
    for c in range(NCORES):
        yc = np.asarray(res.results[c]["y"],
                        dtype=np.float32).reshape(32, 4, 16, 128)
        strip = out[0, :, 32 * c:32 * c + 32, :]
        for p in range(2):
            for q in range(2):
                strip[:, p::2, q::2] = yc[:, 2 * p + q]
    return out
